# revision 1
# baseline (speedup 1.0000x reference)
"""Trainium2 kernel for nn_HSCR_67396626809127 (gnn_message_passing).

The reference network (fc1/fc2 -> 24-step KTD kinematic-tree recurrence ->
cam/pose/shape heads) contains no nonlinearity (dropout is identity in eval
mode), so the whole module is one affine map:

    out[157] = W @ [x(256) | init_pose(144) | init_shape(10) | init_cam(3)] + b

W [157,413] / b [157] are composed on host in float64 from the small weight
tensors (<5MB total), with the bias folded in as a constant-ones feature row
(K = 414).  The device then runs a single data-parallel matmul over the
B*T = 32768 tokens: each of the 8 cores handles 4096 tokens, reading
feature-major activation tiles (transposed on host) and writing a
feature-major output tile that the host transposes back.
"""

import numpy as np

ANCESTOR_INDEX = [[], [0], [0], [0], [0, 1], [0, 2], [0, 3], [0, 1, 4],
                  [0, 2, 5], [0, 3, 6], [0, 1, 4, 7], [0, 2, 5, 8],
                  [0, 3, 6, 9], [0, 3, 6, 9], [0, 3, 6, 9], [0, 3, 6, 9, 12],
                  [0, 3, 6, 9, 13], [0, 3, 6, 9, 14], [0, 3, 6, 9, 13, 16],
                  [0, 3, 6, 9, 14, 17], [0, 3, 6, 9, 13, 16, 18],
                  [0, 3, 6, 9, 14, 17, 19], [0, 3, 6, 9, 13, 16, 18, 20],
                  [0, 3, 6, 9, 14, 17, 19, 21]]
HID = 1024
NCORES = 8
B, T = 2048, 16
NTOK = B * T                 # 32768
TPC = NTOK // NCORES         # 4096 tokens per core
NOUT = 157                   # [cam 3 | pose 144 | shape 10]
KV = 414                     # 413 input features + ones row (bias)
TW = 1024                    # tokens per SBUF tile
NT = TPC // TW               # 4 tiles per core
MCH = [(0, 128), (128, 29)]  # output-feature chunks (psum partition dim)

_PROG = {}


def _compose_affine(fc1_w, fc1_b, fc2_w, fc2_b, decshape_w, decshape_b,
                    deccam_w, deccam_b, ktd_w, ktd_b):
    """Fold the whole network into out = v @ W.T + b, v = [x|pose|shape|cam]."""
    f8 = np.float64
    fc1_w, fc1_b = fc1_w.astype(f8), fc1_b.astype(f8)
    fc2_w, fc2_b = fc2_w.astype(f8), fc2_b.astype(f8)
    decshape_w, decshape_b = decshape_w.astype(f8), decshape_b.astype(f8)
    deccam_w, deccam_b = deccam_w.astype(f8), deccam_b.astype(f8)
    ktd_w, ktd_b = ktd_w.astype(f8), ktd_b.astype(f8)

    F1x, F1s = fc1_w[:, :256], fc1_w[:, 256:266]
    F2x, F2p = fc2_w[:, :256], fc2_w[:, 256:400]

    # KTD recurrence -> pose_out = G @ xc_pose + H @ init_pose + c
    G = np.zeros((24, 6, HID)); H = np.zeros((24, 6, 144)); c = np.zeros((24, 6))
    for j, anc in enumerate(ANCESTOR_INDEX):
        Wj = ktd_w[j]
        G[j] = Wj[:, :HID]
        off = HID
        for i in anc:
            A = Wj[:, off:off + 6]; off += 6
            G[j] += A @ G[i]
            H[j] += A @ H[i]
            c[j] += A @ c[i]
        # reference concatenates init_pose[..., j:j+6] (overlapping slice)
        H[j][:, j:j + 6] += Wj[:, off:off + 6]
        c[j] += ktd_b[j]
    G = G.reshape(144, HID); H = H.reshape(144, 144); c = c.reshape(144)

    Dp, Ds, Dc = deccam_w[:, :HID], deccam_w[:, HID:2 * HID], deccam_w[:, 2 * HID:]

    W = np.zeros((NOUT, 413)); b = np.zeros(NOUT)
    W[0:3, 0:256] = Dp @ F2x + Ds @ F1x
    W[0:3, 256:400] = Dp @ F2p
    W[0:3, 400:410] = Ds @ F1s
    W[0:3, 410:413] = Dc + np.eye(3)
    b[0:3] = Dp @ fc2_b + Ds @ fc1_b + deccam_b

    W[3:147, 0:256] = G @ F2x
    W[3:147, 256:400] = G @ F2p + H + np.eye(144)
    b[3:147] = G @ fc2_b + c

    W[147:157, 0:256] = decshape_w @ F1x
    W[147:157, 400:410] = decshape_w @ F1s + np.eye(10)
    b[147:157] = decshape_w @ fc1_b + decshape_b
    return W.astype(np.float32), b.astype(np.float32)


def _build_program():
    import concourse.bass as bass
    import concourse.tile as tile
    from concourse import bacc, mybir

    f32 = mybir.dt.float32
    f32r = mybir.dt.float32r
    nc = bacc.Bacc("TRN2", target_bir_lowering=False, debug=False,
                   num_devices=NCORES)
    # activations, feature-major: chunks 0..2 packed [128, 3, TPC], chunk 3 [30, TPC]
    # float32r end-to-end: same 4-byte data, PE streams 1 cycle/row vs 4 for f32
    vt012 = nc.declare_dram_parameter("vt012", [128, 3, TPC], f32r, isOutput=False)
    vt3 = nc.declare_dram_parameter("vt3", [30, TPC], f32r, isOutput=False)
    # weights packed [128, 4, NOUT]; chunk 3 rows 30..127 are zero (unused)
    wt = nc.declare_dram_parameter("wt", [128, 4, NOUT], f32r, isOutput=False)
    ot = nc.declare_dram_parameter("ot", [NOUT, TPC], f32, isOutput=True)

    with tile.TileContext(nc) as tc:
        with (
            tc.tile_pool(name="wpool", bufs=1) as wpool,
            tc.tile_pool(name="rhs", bufs=4) as rpool,
            tc.tile_pool(name="outp", bufs=3) as opool,
            tc.tile_pool(name="psum", bufs=4, space=bass.MemorySpace.PSUM) as ppool,
        ):
            w = wpool.tile([128, 4, NOUT], f32r, tag="w", name="w")
            nc.scalar.dma_start(w[:], wt[:])

            for t in range(NT):
                tok = bass.ts(t, TW)
                # alternate the two HWDGE rings (sync=qSP, scalar=qAct) so
                # input streams run on both rings concurrently
                ring = nc.sync if t % 2 == 0 else nc.scalar
                r012 = rpool.tile([128, 3, TW], f32r, tag="r012", name=f"r012_{t}")
                ring.dma_start(r012[:], vt012[:, :, tok])
                r3 = rpool.tile([30, TW], f32r, tag="r3", name=f"r3_{t}")
                ring.dma_start(r3[:], vt3[:, tok])

                otiles = []
                for mi, (m0, dm) in enumerate(MCH):
                    o = opool.tile([dm, TW], f32, tag=f"o{mi}", name=f"o{mi}_{t}")
                    for h in range(TW // 512):
                        hs = bass.ts(h, 512)
                        ps = ppool.tile([dm, 512], f32, tag=f"ps{mi}",
                                        name=f"ps{mi}_{t}_{h}")
                        for k in range(4):
                            if k < 3:
                                lhsT, rhs = w[:, k, m0:m0 + dm], r012[:, k, hs]
                            else:
                                lhsT, rhs = w[0:30, 3, m0:m0 + dm], r3[:, hs]
                            # float32r streams at 1 cycle/row for N>=256
                            # (plain fp32 pays 4x); same 4-byte data
                            nc.tensor.matmul(ps[:], lhsT, rhs,
                                             start=(k == 0), stop=(k == 3))
                        nc.vector.tensor_copy(o[:, hs], ps[:])
                    otiles.append((m0, dm, o))

                # stores go out on gpsimd's SWDGE queues, leaving both
                # HWDGE rings free for the input streams
                for m0, dm, o in otiles:
                    nc.gpsimd.dma_start(ot[m0:m0 + dm, tok], o[:])
    nc.compile()
    return nc


def _get_program():
    if "nc" not in _PROG:
        _PROG["nc"] = _build_program()
    return _PROG["nc"]


def _make_in_maps(x, init_pose, init_shape, init_cam, fc1_w, fc1_b, fc2_w,
                  fc2_b, decshape_w, decshape_b, deccam_w, deccam_b, ktd_w,
                  ktd_b):
    x = np.asarray(x, dtype=np.float32)
    init_pose = np.asarray(init_pose, dtype=np.float32)
    init_shape = np.asarray(init_shape, dtype=np.float32)
    init_cam = np.asarray(init_cam, dtype=np.float32)

    W, b = _compose_affine(
        np.asarray(fc1_w), np.asarray(fc1_b), np.asarray(fc2_w),
        np.asarray(fc2_b), np.asarray(decshape_w), np.asarray(decshape_b),
        np.asarray(deccam_w), np.asarray(deccam_b), np.asarray(ktd_w),
        np.asarray(ktd_b))
    # augment with bias column; device weight layout is [128, 4, 157]
    # (partition p, k-chunk, out-feature), chunk 3 zero-padded past row 30
    W_aug = np.concatenate([W, b[:, None]], axis=1)        # [157, 414]
    wtk = W_aug.T                                           # [414, 157]
    wt = np.zeros((4, 128, NOUT), np.float32)
    wt[0:3] = wtk[0:384].reshape(3, 128, NOUT)
    wt[3, 0:30] = wtk[384:414]
    wt = np.ascontiguousarray(wt.transpose(1, 0, 2))        # [128, 4, 157]

    xs = x.reshape(NCORES, TPC, 256)
    ps = init_pose.reshape(NCORES, TPC, 144)
    ss = init_shape.reshape(NCORES, TPC, 10)
    cs = init_cam.reshape(NCORES, TPC, 3)

    in_maps = []
    for i in range(NCORES):
        v = np.empty((KV, TPC), np.float32)                 # feature-major shard
        v[0:256] = xs[i].T
        v[256:400] = ps[i].T
        v[400:410] = ss[i].T
        v[410:413] = cs[i].T
        v[413] = 1.0
        in_maps.append({
            "vt012": np.ascontiguousarray(
                v[0:384].reshape(3, 128, TPC).transpose(1, 0, 2)),
            "vt3": np.ascontiguousarray(v[384:414]),
            "wt": wt,
        })
    return in_maps


def _assemble(results):
    out_t = np.empty((NOUT, NTOK), np.float32)
    for i in range(NCORES):
        out_t[:, i * TPC:(i + 1) * TPC] = results[i]["ot"]
    return np.ascontiguousarray(out_t.T)


def kernel(x, init_pose, init_shape, init_cam, fc1_w, fc1_b, fc2_w, fc2_b,
           decshape_w, decshape_b, deccam_w, deccam_b, ktd_w, ktd_b):
    from concourse.bass_utils import run_bass_kernel_spmd

    in_maps = _make_in_maps(x, init_pose, init_shape, init_cam, fc1_w, fc1_b,
                            fc2_w, fc2_b, decshape_w, decshape_b, deccam_w,
                            deccam_b, ktd_w, ktd_b)
    nc = _get_program()
    res = run_bass_kernel_spmd(nc, in_maps, list(range(NCORES)))
    return _assemble(res.results)



# revision 2
# speedup vs baseline: 1.5044x; 1.5044x over previous
"""Trainium2 kernel for nn_HSCR_67396626809127 (gnn_message_passing).

The reference network (fc1/fc2 -> 24-step KTD kinematic-tree recurrence ->
cam/pose/shape heads) contains no nonlinearity (dropout is identity in eval
mode), so the whole module is one affine map:

    out[157] = W @ [x(256) | init_pose(144) | init_shape(10) | init_cam(3)] + b

W [157,413] / b [157] are composed on host in float64 from the small weight
tensors (<5MB total), with the bias folded in as a constant-ones feature row
(K = 414).  The device runs a data-parallel matmul over the B*T = 32768
tokens in bf16 (rel tolerance is 2e-2; bf16 end-to-end costs ~3e-3):
each of the 8 cores handles TPC = 4096 tokens.

Device organization (activations-stationary):
  - stationary lhsT = activation tile [128 feats, 128 tokens],
    moving rhs = W^T k-chunk [128 feats, 157 outs]
  - psum [128 tokens, 157 outs] accumulates the 4 k-chunks (K = 414
    packed as 3x128 + 30), so each token column streams ~4.9 PE columns
    instead of 8 (weights-stationary would pay ceil(414/128)*ceil(157/128)).
  - token t of a core maps to (section s, partition p, group q) via
    t = 1024*s + 8*p + q, so each partition's 8 output rows per section
    are contiguous in DRAM (2512B store descriptors) and the output comes
    back token-major: no host-side transpose of the result.
"""

import numpy as np
import ml_dtypes

ANCESTOR_INDEX = [[], [0], [0], [0], [0, 1], [0, 2], [0, 3], [0, 1, 4],
                  [0, 2, 5], [0, 3, 6], [0, 1, 4, 7], [0, 2, 5, 8],
                  [0, 3, 6, 9], [0, 3, 6, 9], [0, 3, 6, 9], [0, 3, 6, 9, 12],
                  [0, 3, 6, 9, 13], [0, 3, 6, 9, 14], [0, 3, 6, 9, 13, 16],
                  [0, 3, 6, 9, 14, 17], [0, 3, 6, 9, 13, 16, 18],
                  [0, 3, 6, 9, 14, 17, 19], [0, 3, 6, 9, 13, 16, 18, 20],
                  [0, 3, 6, 9, 14, 17, 19, 21]]
HID = 1024
NCORES = 8
B, T = 2048, 16
NTOK = B * T                 # 32768
TPC = NTOK // NCORES         # 4096 tokens per core
NOUT = 157                   # [cam 3 | pose 144 | shape 10]
KV = 414                     # 413 input features + ones row (bias)
NSEC = 4                     # sections of 1024 tokens
NGRP = 8                     # psum groups per section (token = 1024s+8p+q)

_PROG = {}


def _compose_affine(fc1_w, fc1_b, fc2_w, fc2_b, decshape_w, decshape_b,
                    deccam_w, deccam_b, ktd_w, ktd_b):
    """Fold the whole network into out = v @ W.T + b, v = [x|pose|shape|cam]."""
    f8 = np.float64
    fc1_w, fc1_b = fc1_w.astype(f8), fc1_b.astype(f8)
    fc2_w, fc2_b = fc2_w.astype(f8), fc2_b.astype(f8)
    decshape_w, decshape_b = decshape_w.astype(f8), decshape_b.astype(f8)
    deccam_w, deccam_b = deccam_w.astype(f8), deccam_b.astype(f8)
    ktd_w, ktd_b = ktd_w.astype(f8), ktd_b.astype(f8)

    F1x, F1s = fc1_w[:, :256], fc1_w[:, 256:266]
    F2x, F2p = fc2_w[:, :256], fc2_w[:, 256:400]

    # KTD recurrence -> pose_out = G @ xc_pose + H @ init_pose + c
    G = np.zeros((24, 6, HID)); H = np.zeros((24, 6, 144)); c = np.zeros((24, 6))
    for j, anc in enumerate(ANCESTOR_INDEX):
        Wj = ktd_w[j]
        G[j] = Wj[:, :HID]
        off = HID
        for i in anc:
            A = Wj[:, off:off + 6]; off += 6
            G[j] += A @ G[i]
            H[j] += A @ H[i]
            c[j] += A @ c[i]
        # reference concatenates init_pose[..., j:j+6] (overlapping slice)
        H[j][:, j:j + 6] += Wj[:, off:off + 6]
        c[j] += ktd_b[j]
    G = G.reshape(144, HID); H = H.reshape(144, 144); c = c.reshape(144)

    Dp, Ds, Dc = deccam_w[:, :HID], deccam_w[:, HID:2 * HID], deccam_w[:, 2 * HID:]

    W = np.zeros((NOUT, 413)); b = np.zeros(NOUT)
    W[0:3, 0:256] = Dp @ F2x + Ds @ F1x
    W[0:3, 256:400] = Dp @ F2p
    W[0:3, 400:410] = Ds @ F1s
    W[0:3, 410:413] = Dc + np.eye(3)
    b[0:3] = Dp @ fc2_b + Ds @ fc1_b + deccam_b

    W[3:147, 0:256] = G @ F2x
    W[3:147, 256:400] = G @ F2p + H + np.eye(144)
    b[3:147] = G @ fc2_b + c

    W[147:157, 0:256] = decshape_w @ F1x
    W[147:157, 400:410] = decshape_w @ F1s + np.eye(10)
    b[147:157] = decshape_w @ fc1_b + decshape_b
    return W.astype(np.float32), b.astype(np.float32)


def _build_program():
    import concourse.bass as bass
    import concourse.tile as tile
    from concourse import bacc, mybir

    f32 = mybir.dt.float32
    bf16 = mybir.dt.bfloat16
    nc = bacc.Bacc("TRN2", target_bir_lowering=False, debug=False,
                   num_devices=NCORES)
    # activations, feature-major: feats 0..383 in [128, 3, TPC], tail in [30, TPC]
    vt012 = nc.declare_dram_parameter("vt012", [128, 3, TPC], bf16, isOutput=False)
    vt3 = nc.declare_dram_parameter("vt3", [30, TPC], bf16, isOutput=False)
    # W^T packed [128, 4, NOUT]; chunk 3 rows 30..127 are zero (unused)
    wt = nc.declare_dram_parameter("wt", [128, 4, NOUT], bf16, isOutput=False)
    # token-major output
    ot = nc.declare_dram_parameter("ot", [TPC, NOUT], bf16, isOutput=True)

    with tile.TileContext(nc) as tc:
        with (
            tc.tile_pool(name="wpool", bufs=1) as wpool,
            tc.tile_pool(name="rin", bufs=3) as rpool,
            tc.tile_pool(name="outp", bufs=3) as opool,
            tc.tile_pool(name="psum", bufs=1, space=bass.MemorySpace.PSUM) as ppool,
        ):
            w = wpool.tile([128, 4, NOUT], bf16, tag="w", name="w")
            nc.sync.dma_start(w[:], wt[:])

            r012s, r3s = [], []

            def load_section(s):
                ring = nc.sync if s % 2 == 0 else nc.scalar
                tok = slice(1024 * s, 1024 * (s + 1))
                # view [128, 3, 128, 8]: free idx = chunk*1024 + 8*p' + q
                r012 = rpool.tile([128, 3, 128, NGRP], bf16, tag="r012",
                                  name=f"r012_{s}")
                if s == 0:
                    # per-chunk loads so the first matmuls start sooner
                    for k in range(3):
                        ring = nc.sync if k % 2 == 0 else nc.scalar
                        ring.dma_start(r012[:, k], vt012[:, k, tok])
                    ring = nc.scalar
                else:
                    ring.dma_start(r012[:], vt012[:, :, tok])
                r3 = rpool.tile([30, 128, NGRP], bf16, tag="r3", name=f"r3_{s}")
                ring.dma_start(r3[:], vt3[:, tok])
                r012s.append(r012)
                r3s.append(r3)

            load_section(0)
            load_section(1)

            for s in range(NSEC):
                if s + 2 < NSEC:
                    load_section(s + 2)
                r012, r3 = r012s[s], r3s[s]
                pss = []
                for q in range(NGRP):
                    ps = ppool.tile([128, 512], f32, tag=f"ps{q}",
                                    name=f"ps_{s}_{q}")
                    pss.append(ps)
                for k in range(4):
                    rhs = w[:, k, :] if k < 3 else w[0:30, 3, :]
                    for q in range(NGRP):
                        lhsT = r012[:, k, :, q] if k < 3 else r3[:, :, q]
                        nc.tensor.matmul(pss[q][:, 0:NOUT], lhsT, rhs,
                                         start=(k == 0), stop=(k == 3))
                outt = opool.tile([128, NGRP, NOUT], bf16, tag="out",
                                  name=f"out_{s}")
                for q in range(NGRP):
                    if q % 2 == 0:
                        nc.vector.tensor_copy(outt[:, q, :], pss[q][:, 0:NOUT])
                    else:
                        nc.scalar.copy(outt[:, q, :], pss[q][:, 0:NOUT])
                nc.gpsimd.dma_start(ot[1024 * s:1024 * (s + 1), :], outt[:])
    nc.compile()
    return nc


def _get_program():
    if "nc" not in _PROG:
        _PROG["nc"] = _build_program()
    return _PROG["nc"]


def _make_in_maps(x, init_pose, init_shape, init_cam, fc1_w, fc1_b, fc2_w,
                  fc2_b, decshape_w, decshape_b, deccam_w, deccam_b, ktd_w,
                  ktd_b):
    bf = ml_dtypes.bfloat16
    x = np.asarray(x, dtype=np.float32)
    init_pose = np.asarray(init_pose, dtype=np.float32)
    init_shape = np.asarray(init_shape, dtype=np.float32)
    init_cam = np.asarray(init_cam, dtype=np.float32)

    W, b = _compose_affine(
        np.asarray(fc1_w), np.asarray(fc1_b), np.asarray(fc2_w),
        np.asarray(fc2_b), np.asarray(decshape_w), np.asarray(decshape_b),
        np.asarray(deccam_w), np.asarray(deccam_b), np.asarray(ktd_w),
        np.asarray(ktd_b))
    # augment with bias column; device weight layout is [128, 4, 157]
    # (partition p, k-chunk, out-feature), chunk 3 zero-padded past row 30
    W_aug = np.concatenate([W, b[:, None]], axis=1)        # [157, 414]
    wtk = W_aug.T.astype(bf)                                # [414, 157]
    wt = np.zeros((128, 4, NOUT), bf)
    wt[:, 0:3, :] = wtk[0:384].reshape(3, 128, NOUT).transpose(1, 0, 2)
    wt[0:30, 3, :] = wtk[384:414]
    wt = np.ascontiguousarray(wt)

    xs = x.reshape(NCORES, TPC, 256)
    ps = init_pose.reshape(NCORES, TPC, 144)
    ss = init_shape.reshape(NCORES, TPC, 10)
    cs = init_cam.reshape(NCORES, TPC, 3)

    in_maps = []
    for i in range(NCORES):
        v = np.empty((KV, TPC), np.float32)                 # feature-major shard
        v[0:256] = xs[i].T
        v[256:400] = ps[i].T
        v[400:410] = ss[i].T
        v[410:413] = cs[i].T
        v[413] = 1.0
        vb = v.astype(bf)
        in_maps.append({
            "vt012": np.ascontiguousarray(
                vb[0:384].reshape(3, 128, TPC).transpose(1, 0, 2)),
            "vt3": np.ascontiguousarray(vb[384:414]),
            "wt": wt,
        })
    return in_maps


def _assemble(results):
    out = np.empty((NTOK, NOUT), np.float32)
    for i in range(NCORES):
        out[i * TPC:(i + 1) * TPC] = results[i]["ot"].astype(np.float32)
    return out


def kernel(x, init_pose, init_shape, init_cam, fc1_w, fc1_b, fc2_w, fc2_b,
           decshape_w, decshape_b, deccam_w, deccam_b, ktd_w, ktd_b):
    from concourse.bass_utils import run_bass_kernel_spmd

    in_maps = _make_in_maps(x, init_pose, init_shape, init_cam, fc1_w, fc1_b,
                            fc2_w, fc2_b, decshape_w, decshape_b, deccam_w,
                            deccam_b, ktd_w, ktd_b)
    nc = _get_program()
    res = run_bass_kernel_spmd(nc, in_maps, list(range(NCORES)))
    return _assemble(res.results)


# revision 3
# speedup vs baseline: 1.5076x; 1.0021x over previous
"""Trainium2 kernel for nn_HSCR_67396626809127 (gnn_message_passing).

The reference network (fc1/fc2 -> 24-step KTD kinematic-tree recurrence ->
cam/pose/shape heads) contains no nonlinearity (dropout is identity in eval
mode), so the whole module is one affine map:

    out[157] = W @ [x(256) | init_pose(144) | init_shape(10) | init_cam(3)] + b

W [157,413] / b [157] are composed on host in float64 from the small weight
tensors (<5MB total), with the bias folded in as a constant-ones feature row
(K = 414).  The device runs a data-parallel matmul over the B*T = 32768
tokens in bf16 (rel tolerance is 2e-2; bf16 end-to-end costs ~3e-3):
each of the 8 cores handles TPC = 4096 tokens.

Device organization (activations-stationary):
  - stationary lhsT = activation tile [128 feats, 128 tokens],
    moving rhs = W^T k-chunk [128 feats, 157 outs]
  - psum [128 tokens, 157 outs] accumulates the 4 k-chunks (K = 414
    packed as 3x128 + 30), so each token column streams ~4.9 PE columns
    instead of 8 (weights-stationary would pay ceil(414/128)*ceil(157/128)).
  - token t of a core maps to (section s, partition p, group q) via
    t = 1024*s + 8*p + q; input DRAM is packed per-section-contiguous
    (6KB DMA descriptors) and the output is stored in (p, half, s', q)
    order so each half-store writes 5KB contiguous runs per partition.
  - a short burst of warm-up matmuls on a memset tile runs right after
    the engine preamble so the PE HAM throttle reaches full clock before
    the real matmul stream begins.
"""

import numpy as np
import ml_dtypes

ANCESTOR_INDEX = [[], [0], [0], [0], [0, 1], [0, 2], [0, 3], [0, 1, 4],
                  [0, 2, 5], [0, 3, 6], [0, 1, 4, 7], [0, 2, 5, 8],
                  [0, 3, 6, 9], [0, 3, 6, 9], [0, 3, 6, 9], [0, 3, 6, 9, 12],
                  [0, 3, 6, 9, 13], [0, 3, 6, 9, 14], [0, 3, 6, 9, 13, 16],
                  [0, 3, 6, 9, 14, 17], [0, 3, 6, 9, 13, 16, 18],
                  [0, 3, 6, 9, 14, 17, 19], [0, 3, 6, 9, 13, 16, 18, 20],
                  [0, 3, 6, 9, 14, 17, 19, 21]]
HID = 1024
NCORES = 8
B, T = 2048, 16
NTOK = B * T                 # 32768
TPC = NTOK // NCORES         # 4096 tokens per core
NOUT = 157                   # [cam 3 | pose 144 | shape 10]
KV = 414                     # 413 input features + ones row (bias)
NSEC = 4                     # sections of 1024 tokens
NGRP = 8                     # psum groups per section (token = 1024s+8p+q)
NWARM = 5                    # warm-up matmuls (N=512) before the real stream

_PROG = {}


def _compose_affine(fc1_w, fc1_b, fc2_w, fc2_b, decshape_w, decshape_b,
                    deccam_w, deccam_b, ktd_w, ktd_b):
    """Fold the whole network into out = v @ W.T + b, v = [x|pose|shape|cam]."""
    f8 = np.float64
    fc1_w, fc1_b = fc1_w.astype(f8), fc1_b.astype(f8)
    fc2_w, fc2_b = fc2_w.astype(f8), fc2_b.astype(f8)
    decshape_w, decshape_b = decshape_w.astype(f8), decshape_b.astype(f8)
    deccam_w, deccam_b = deccam_w.astype(f8), deccam_b.astype(f8)
    ktd_w, ktd_b = ktd_w.astype(f8), ktd_b.astype(f8)

    F1x, F1s = fc1_w[:, :256], fc1_w[:, 256:266]
    F2x, F2p = fc2_w[:, :256], fc2_w[:, 256:400]

    # KTD recurrence -> pose_out = G @ xc_pose + H @ init_pose + c
    G = np.zeros((24, 6, HID)); H = np.zeros((24, 6, 144)); c = np.zeros((24, 6))
    for j, anc in enumerate(ANCESTOR_INDEX):
        Wj = ktd_w[j]
        G[j] = Wj[:, :HID]
        off = HID
        for i in anc:
            A = Wj[:, off:off + 6]; off += 6
            G[j] += A @ G[i]
            H[j] += A @ H[i]
            c[j] += A @ c[i]
        # reference concatenates init_pose[..., j:j+6] (overlapping slice)
        H[j][:, j:j + 6] += Wj[:, off:off + 6]
        c[j] += ktd_b[j]
    G = G.reshape(144, HID); H = H.reshape(144, 144); c = c.reshape(144)

    Dp, Ds, Dc = deccam_w[:, :HID], deccam_w[:, HID:2 * HID], deccam_w[:, 2 * HID:]

    W = np.zeros((NOUT, 413)); b = np.zeros(NOUT)
    W[0:3, 0:256] = Dp @ F2x + Ds @ F1x
    W[0:3, 256:400] = Dp @ F2p
    W[0:3, 400:410] = Ds @ F1s
    W[0:3, 410:413] = Dc + np.eye(3)
    b[0:3] = Dp @ fc2_b + Ds @ fc1_b + deccam_b

    W[3:147, 0:256] = G @ F2x
    W[3:147, 256:400] = G @ F2p + H + np.eye(144)
    b[3:147] = G @ fc2_b + c

    W[147:157, 0:256] = decshape_w @ F1x
    W[147:157, 400:410] = decshape_w @ F1s + np.eye(10)
    b[147:157] = decshape_w @ fc1_b + decshape_b
    return W.astype(np.float32), b.astype(np.float32)


def _build_program():
    import concourse.bass as bass
    import concourse.tile as tile
    from concourse import bacc, mybir

    f32 = mybir.dt.float32
    bf16 = mybir.dt.bfloat16
    nc = bacc.Bacc("TRN2", target_bir_lowering=False, debug=False,
                   num_devices=NCORES)
    # activations packed per section: vtp[s, f, c, p, q] = feature (c*128+f)
    # of token (1024s + 8p + q) -- 6KB contiguous per partition per section
    vtp = nc.declare_dram_parameter("vtp", [NSEC, 128, 3, 128, NGRP], bf16,
                                    isOutput=False)
    vt3p = nc.declare_dram_parameter("vt3p", [NSEC, 30, 128, NGRP], bf16,
                                     isOutput=False)
    # W^T packed [128, 4, NOUT]; chunk 3 rows 30..127 are zero (unused)
    wt = nc.declare_dram_parameter("wt", [128, 4, NOUT], bf16, isOutput=False)
    # output in (p, half, s', q, o) order; host un-permutes
    ot = nc.declare_dram_parameter("ot", [128, 2, 2, NGRP, NOUT], bf16,
                                   isOutput=True)

    with tile.TileContext(nc) as tc:
        with (
            tc.tile_pool(name="wpool", bufs=1) as wpool,
            tc.tile_pool(name="rin", bufs=3) as rpool,
            tc.tile_pool(name="outp", bufs=2) as opool,
            tc.tile_pool(name="psum", bufs=1, space=bass.MemorySpace.PSUM) as ppool,
        ):
            # PE warm-up: memset a zeros tile, run a few N=512 matmuls into
            # the ps0 slot so the HAM throttle sees sustained PE activity
            # while the first input DMAs are still in flight.
            z = wpool.tile([128, 512], bf16, tag="z", name="z")
            nc.gpsimd.memset(z[:], 0.0)
            psw = ppool.tile([128, 512], f32, tag="ps0", name="ps_warm")
            for i in range(NWARM):
                nc.tensor.matmul(psw[:], z[:, 0:128], z[:],
                                 start=(i == 0), stop=(i == NWARM - 1))
            zsink = wpool.tile([128, 512], bf16, tag="zsink", name="zsink")
            nc.vector.tensor_copy(zsink[:], psw[:])

            w = wpool.tile([128, 4, NOUT], bf16, tag="w", name="w")

            # input loads: section 0 split per-chunk so the first matmul can
            # start as soon as chunk 0 lands; weights on the scalar ring
            r012s, r3s = [], []
            r012_0 = rpool.tile([128, 3, 128, NGRP], bf16, tag="r012",
                                name="r012_0")
            nc.sync.dma_start(r012_0[:, 0], vtp[0, :, 0])
            nc.scalar.dma_start(w[:], wt[:])
            nc.scalar.dma_start(r012_0[:, 1:3], vtp[0, :, 1:3])
            r3_0 = rpool.tile([30, 128, NGRP], bf16, tag="r3", name="r3_0")
            nc.scalar.dma_start(r3_0[:], vt3p[0])
            r012s.append(r012_0); r3s.append(r3_0)

            def load_section(s):
                ring = nc.sync if s % 2 == 1 else nc.scalar
                r012 = rpool.tile([128, 3, 128, NGRP], bf16, tag="r012",
                                  name=f"r012_{s}")
                ring.dma_start(r012[:], vtp[s])
                r3 = rpool.tile([30, 128, NGRP], bf16, tag="r3", name=f"r3_{s}")
                ring.dma_start(r3[:], vt3p[s])
                r012s.append(r012)
                r3s.append(r3)

            load_section(1)

            for s in range(NSEC):
                if s + 1 < NSEC and s > 0:
                    load_section(s + 1)
                r012, r3 = r012s[s], r3s[s]
                pss = []
                for q in range(NGRP):
                    ps = ppool.tile([128, 512], f32, tag=f"ps{q}",
                                    name=f"ps_{s}_{q}")
                    pss.append(ps)
                for k in range(4):
                    rhs = w[:, k, :] if k < 3 else w[0:30, 3, :]
                    for q in range(NGRP):
                        lhsT = r012[:, k, :, q] if k < 3 else r3[:, :, q]
                        nc.tensor.matmul(pss[q][:, 0:NOUT], lhsT, rhs,
                                         start=(k == 0), stop=(k == 3))
                h, sh = divmod(s, 2)
                if sh == 0:
                    outt = opool.tile([128, 2, NGRP, NOUT], bf16, tag="out",
                                      name=f"out_{h}")
                for q in range(NGRP):
                    if q % 2 == 0:
                        nc.vector.tensor_copy(outt[:, sh, q, :],
                                              pss[q][:, 0:NOUT])
                    else:
                        nc.scalar.copy(outt[:, sh, q, :], pss[q][:, 0:NOUT])
                if sh == 1:
                    nc.gpsimd.dma_start(ot[:, h], outt[:])
    nc.compile()
    return nc


def _get_program():
    if "nc" not in _PROG:
        _PROG["nc"] = _build_program()
    return _PROG["nc"]


def _make_in_maps(x, init_pose, init_shape, init_cam, fc1_w, fc1_b, fc2_w,
                  fc2_b, decshape_w, decshape_b, deccam_w, deccam_b, ktd_w,
                  ktd_b):
    bf = ml_dtypes.bfloat16
    x = np.asarray(x, dtype=np.float32)
    init_pose = np.asarray(init_pose, dtype=np.float32)
    init_shape = np.asarray(init_shape, dtype=np.float32)
    init_cam = np.asarray(init_cam, dtype=np.float32)

    W, b = _compose_affine(
        np.asarray(fc1_w), np.asarray(fc1_b), np.asarray(fc2_w),
        np.asarray(fc2_b), np.asarray(decshape_w), np.asarray(decshape_b),
        np.asarray(deccam_w), np.asarray(deccam_b), np.asarray(ktd_w),
        np.asarray(ktd_b))
    # augment with bias column; device weight layout is [128, 4, 157]
    # (partition p, k-chunk, out-feature), chunk 3 zero-padded past row 30
    W_aug = np.concatenate([W, b[:, None]], axis=1)        # [157, 414]
    wtk = W_aug.T.astype(bf)                                # [414, 157]
    wt = np.zeros((128, 4, NOUT), bf)
    wt[:, 0:3, :] = wtk[0:384].reshape(3, 128, NOUT).transpose(1, 0, 2)
    wt[0:30, 3, :] = wtk[384:414]
    wt = np.ascontiguousarray(wt)

    xs = x.reshape(NCORES, TPC, 256)
    ps = init_pose.reshape(NCORES, TPC, 144)
    ss = init_shape.reshape(NCORES, TPC, 10)
    cs = init_cam.reshape(NCORES, TPC, 3)

    in_maps = []
    for i in range(NCORES):
        v = np.empty((KV, TPC), np.float32)                 # feature-major shard
        v[0:256] = xs[i].T
        v[256:400] = ps[i].T
        v[400:410] = ss[i].T
        v[410:413] = cs[i].T
        v[413] = 1.0
        vb = v.astype(bf)
        # vtp[s, f, c, p, q] = v[c*128+f, 1024s+8p+q]
        vtp = vb[0:384].reshape(3, 128, NSEC, 128, NGRP).transpose(2, 1, 0, 3, 4)
        vt3p = vb[384:414].reshape(30, NSEC, 128, NGRP).transpose(1, 0, 2, 3)
        in_maps.append({
            "vtp": np.ascontiguousarray(vtp),
            "vt3p": np.ascontiguousarray(vt3p),
            "wt": wt,
        })
    return in_maps


def _assemble(results):
    out = np.empty((NTOK, NOUT), np.float32)
    for i in range(NCORES):
        # ot[p, h, s', q, o] -> token 1024*(2h+s') + 8p + q
        o = results[i]["ot"].astype(np.float32).reshape(128, NSEC, NGRP, NOUT)
        out[i * TPC:(i + 1) * TPC] = (
            o.transpose(1, 0, 2, 3).reshape(TPC, NOUT))
    return out


def kernel(x, init_pose, init_shape, init_cam, fc1_w, fc1_b, fc2_w, fc2_b,
           decshape_w, decshape_b, deccam_w, deccam_b, ktd_w, ktd_b):
    from concourse.bass_utils import run_bass_kernel_spmd

    in_maps = _make_in_maps(x, init_pose, init_shape, init_cam, fc1_w, fc1_b,
                            fc2_w, fc2_b, decshape_w, decshape_b, deccam_w,
                            deccam_b, ktd_w, ktd_b)
    nc = _get_program()
    res = run_bass_kernel_spmd(nc, in_maps, list(range(NCORES)))
    return _assemble(res.results)


# revision 9
# speedup vs baseline: 1.7436x; 1.1566x over previous
"""Trainium2 kernel for nn_HSCR_67396626809127 (gnn_message_passing).

The reference network (fc1/fc2 -> 24-step KTD kinematic-tree recurrence ->
cam/pose/shape heads) contains no nonlinearity (dropout is identity in eval
mode), so the whole module is one affine map:

    out[157] = W @ [x(256) | init_pose(144) | init_shape(10) | init_cam(3)] + b

W [157,413] / b [157] are composed on host in float64 from the small weight
tensors (<5MB total), with the bias folded in as a constant-ones feature row
(K = 414).  The device runs a data-parallel matmul over the B*T = 32768
tokens in bf16 (rel tolerance is 2e-2; bf16 end-to-end costs ~3e-3):
each of the 8 cores handles TPC = 4096 tokens.

Device organization (activations-stationary):
  - stationary lhsT = activation tile [128 feats, 128 tokens],
    moving rhs = W^T k-chunk [128 feats, 157 outs]
  - psum [128 tokens, 157 outs] accumulates the 4 k-chunks (K = 414
    packed as 3x128 + 30), so each token column streams ~4.9 PE columns
    instead of 8 (weights-stationary would pay ceil(414/128)*ceil(157/128)).
  - token t of a core maps to (section s, partition p, group q) via
    t = 1024*s + 8*p + q; input DRAM is packed per-section-contiguous
    (6KB DMA descriptors) and the output is stored in (p, half, s', q)
    order so each half-store writes 5KB contiguous runs per partition.
  - a short burst of warm-up matmuls on a memset tile runs right after
    the engine preamble so the PE HAM throttle reaches full clock before
    the real matmul stream begins.
"""

import numpy as np
import ml_dtypes

ANCESTOR_INDEX = [[], [0], [0], [0], [0, 1], [0, 2], [0, 3], [0, 1, 4],
                  [0, 2, 5], [0, 3, 6], [0, 1, 4, 7], [0, 2, 5, 8],
                  [0, 3, 6, 9], [0, 3, 6, 9], [0, 3, 6, 9], [0, 3, 6, 9, 12],
                  [0, 3, 6, 9, 13], [0, 3, 6, 9, 14], [0, 3, 6, 9, 13, 16],
                  [0, 3, 6, 9, 14, 17], [0, 3, 6, 9, 13, 16, 18],
                  [0, 3, 6, 9, 14, 17, 19], [0, 3, 6, 9, 13, 16, 18, 20],
                  [0, 3, 6, 9, 14, 17, 19, 21]]
HID = 1024
NCORES = 8
B, T = 2048, 16
NTOK = B * T                 # 32768
TPC = NTOK // NCORES         # 4096 tokens per core
NOUT = 157                   # [cam 3 | pose 144 | shape 10]
KV = 414                     # 413 input features + ones row (bias)
NSEC = 4                     # sections of 1024 tokens
NGRP = 8                     # psum groups per section (token = 1024s+8p+q)
NWARM = 5                    # warm-up matmuls (N=512) before the real stream

_PROG = {}


def _compose_affine(fc1_w, fc1_b, fc2_w, fc2_b, decshape_w, decshape_b,
                    deccam_w, deccam_b, ktd_w, ktd_b):
    """Fold the whole network into out = v @ W.T + b, v = [x|pose|shape|cam]."""
    f8 = np.float64
    fc1_w, fc1_b = fc1_w.astype(f8), fc1_b.astype(f8)
    fc2_w, fc2_b = fc2_w.astype(f8), fc2_b.astype(f8)
    decshape_w, decshape_b = decshape_w.astype(f8), decshape_b.astype(f8)
    deccam_w, deccam_b = deccam_w.astype(f8), deccam_b.astype(f8)
    ktd_w, ktd_b = ktd_w.astype(f8), ktd_b.astype(f8)

    F1x, F1s = fc1_w[:, :256], fc1_w[:, 256:266]
    F2x, F2p = fc2_w[:, :256], fc2_w[:, 256:400]

    # KTD recurrence -> pose_out = G @ xc_pose + H @ init_pose + c
    G = np.zeros((24, 6, HID)); H = np.zeros((24, 6, 144)); c = np.zeros((24, 6))
    for j, anc in enumerate(ANCESTOR_INDEX):
        Wj = ktd_w[j]
        G[j] = Wj[:, :HID]
        off = HID
        for i in anc:
            A = Wj[:, off:off + 6]; off += 6
            G[j] += A @ G[i]
            H[j] += A @ H[i]
            c[j] += A @ c[i]
        # reference concatenates init_pose[..., j:j+6] (overlapping slice)
        H[j][:, j:j + 6] += Wj[:, off:off + 6]
        c[j] += ktd_b[j]
    G = G.reshape(144, HID); H = H.reshape(144, 144); c = c.reshape(144)

    Dp, Ds, Dc = deccam_w[:, :HID], deccam_w[:, HID:2 * HID], deccam_w[:, 2 * HID:]

    W = np.zeros((NOUT, 413)); b = np.zeros(NOUT)
    W[0:3, 0:256] = Dp @ F2x + Ds @ F1x
    W[0:3, 256:400] = Dp @ F2p
    W[0:3, 400:410] = Ds @ F1s
    W[0:3, 410:413] = Dc + np.eye(3)
    b[0:3] = Dp @ fc2_b + Ds @ fc1_b + deccam_b

    W[3:147, 0:256] = G @ F2x
    W[3:147, 256:400] = G @ F2p + H + np.eye(144)
    b[3:147] = G @ fc2_b + c

    W[147:157, 0:256] = decshape_w @ F1x
    W[147:157, 400:410] = decshape_w @ F1s + np.eye(10)
    b[147:157] = decshape_w @ fc1_b + decshape_b
    return W.astype(np.float32), b.astype(np.float32)


def _build_program():
    import concourse.bass as bass
    import concourse.tile as tile
    from concourse import bacc, mybir

    f32 = mybir.dt.float32
    bf16 = mybir.dt.bfloat16
    nc = bacc.Bacc("TRN2", target_bir_lowering=False, debug=False,
                   num_devices=NCORES)
    # activations packed per section: vtp[s, f, c, q, p] = feature (c*128+f)
    # of token (1024s + 8p + q) -- 6KB contiguous per partition per section,
    # and lhsT slices [:, k, q, :] are contiguous (enables PE fast weight load)
    vtp = nc.declare_dram_parameter("vtp", [NSEC, 128, 3, NGRP, 128], bf16,
                                    isOutput=False)
    vt3p = nc.declare_dram_parameter("vt3p", [NSEC, 30, NGRP, 128], bf16,
                                     isOutput=False)
    # W^T packed [128, 4, NOUT]; chunk 3 rows 30..127 are zero (unused)
    wt = nc.declare_dram_parameter("wt", [128, 4, NOUT], bf16, isOutput=False)
    # output in (p, half, s', q, o) order; host un-permutes
    ot = nc.declare_dram_parameter("ot", [128, 2, 2, NGRP, NOUT], bf16,
                                   isOutput=True)

    with tile.TileContext(nc) as tc:
        with (
            tc.tile_pool(name="wpool", bufs=1) as wpool,
            tc.tile_pool(name="rin", bufs=3) as rpool,
            tc.tile_pool(name="outp", bufs=2) as opool,
            tc.tile_pool(name="psum", bufs=1, space=bass.MemorySpace.PSUM) as ppool,
        ):
            # PE warm-up: memset a zeros tile, run a few N=512 matmuls into
            # the ps0 slot so the HAM throttle sees sustained PE activity
            # while the first input DMAs are still in flight.
            z = wpool.tile([128, 512], bf16, tag="z", name="z")
            nc.gpsimd.memset(z[:], 0.0)
            psw = ppool.tile([128, 512], f32, tag="ps0", name="ps_warm")
            for i in range(NWARM):
                nc.tensor.matmul(psw[:], z[:, 0:128], z[:],
                                 start=(i == 0), stop=(i == NWARM - 1))
            zsink = wpool.tile([128, 512], bf16, tag="zsink", name="zsink")
            nc.vector.tensor_copy(zsink[:], psw[:])

            w = wpool.tile([128, 4, NOUT], bf16, tag="w", name="w")

            # input loads: section 0 split per-chunk so the first matmul can
            # start as soon as chunk 0 lands
            r012s, r3s = [], []
            r012_0 = rpool.tile([128, 3, NGRP, 128], bf16, tag="r012",
                                name="r012_0")
            nc.sync.dma_start(r012_0[:, 0], vtp[0, :, 0])
            nc.scalar.dma_start(w[:], wt[:])
            nc.sync.dma_start(r012_0[:, 1], vtp[0, :, 1])
            nc.scalar.dma_start(r012_0[:, 2], vtp[0, :, 2])
            r3_0 = rpool.tile([30, NGRP, 128], bf16, tag="r3", name="r3_0")
            nc.scalar.dma_start(r3_0[:], vt3p[0])
            r012s.append(r012_0); r3s.append(r3_0)

            def load_section(s):
                ring = nc.sync if s % 2 == 1 else nc.scalar
                r012 = rpool.tile([128, 3, NGRP, 128], bf16, tag="r012",
                                  name=f"r012_{s}")
                ring.dma_start(r012[:], vtp[s])
                r3 = rpool.tile([30, NGRP, 128], bf16, tag="r3", name=f"r3_{s}")
                ring.dma_start(r3[:], vt3p[s])
                r012s.append(r012)
                r3s.append(r3)

            load_section(1)

            for s in range(NSEC):
                if s + 1 < NSEC and s > 0:
                    load_section(s + 1)
                r012, r3 = r012s[s], r3s[s]
                pss = []
                for q in range(NGRP):
                    ps = ppool.tile([128, 512], f32, tag=f"ps{q}",
                                    name=f"ps_{s}_{q}")
                    pss.append(ps)
                for k in range(4):
                    rhs = w[:, k, :] if k < 3 else w[0:30, 3, :]
                    for q in range(NGRP):
                        lhsT = r012[:, k, q, :] if k < 3 else r3[:, q, :]
                        nc.tensor.matmul(pss[q][:, 0:NOUT], lhsT, rhs,
                                         start=(k == 0), stop=(k == 3))
                h, sh = divmod(s, 2)
                if sh == 0:
                    outt = opool.tile([128, 2, NGRP, NOUT], bf16, tag="out",
                                      name=f"out_{h}")
                for q in range(NGRP):
                    if q % 2 == 0:
                        nc.vector.tensor_copy(outt[:, sh, q, :],
                                              pss[q][:, 0:NOUT])
                    else:
                        nc.scalar.copy(outt[:, sh, q, :], pss[q][:, 0:NOUT])
                if sh == 1:
                    nc.gpsimd.dma_start(ot[:, h], outt[:])
    nc.compile()
    return nc


def _get_program():
    if "nc" not in _PROG:
        _PROG["nc"] = _build_program()
    return _PROG["nc"]


def _make_in_maps(x, init_pose, init_shape, init_cam, fc1_w, fc1_b, fc2_w,
                  fc2_b, decshape_w, decshape_b, deccam_w, deccam_b, ktd_w,
                  ktd_b):
    bf = ml_dtypes.bfloat16
    x = np.asarray(x, dtype=np.float32)
    init_pose = np.asarray(init_pose, dtype=np.float32)
    init_shape = np.asarray(init_shape, dtype=np.float32)
    init_cam = np.asarray(init_cam, dtype=np.float32)

    W, b = _compose_affine(
        np.asarray(fc1_w), np.asarray(fc1_b), np.asarray(fc2_w),
        np.asarray(fc2_b), np.asarray(decshape_w), np.asarray(decshape_b),
        np.asarray(deccam_w), np.asarray(deccam_b), np.asarray(ktd_w),
        np.asarray(ktd_b))
    # augment with bias column; device weight layout is [128, 4, 157]
    # (partition p, k-chunk, out-feature), chunk 3 zero-padded past row 30
    W_aug = np.concatenate([W, b[:, None]], axis=1)        # [157, 414]
    wtk = W_aug.T.astype(bf)                                # [414, 157]
    wt = np.zeros((128, 4, NOUT), bf)
    wt[:, 0:3, :] = wtk[0:384].reshape(3, 128, NOUT).transpose(1, 0, 2)
    wt[0:30, 3, :] = wtk[384:414]
    wt = np.ascontiguousarray(wt)

    xs = x.reshape(NCORES, TPC, 256)
    ps = init_pose.reshape(NCORES, TPC, 144)
    ss = init_shape.reshape(NCORES, TPC, 10)
    cs = init_cam.reshape(NCORES, TPC, 3)

    in_maps = []
    for i in range(NCORES):
        v = np.empty((KV, TPC), np.float32)                 # feature-major shard
        v[0:256] = xs[i].T
        v[256:400] = ps[i].T
        v[400:410] = ss[i].T
        v[410:413] = cs[i].T
        v[413] = 1.0
        vb = v.astype(bf)
        # vtp[s, f, c, q, p] = v[c*128+f, 1024s+8p+q]
        vtp = vb[0:384].reshape(3, 128, NSEC, 128, NGRP).transpose(2, 1, 0, 4, 3)
        vt3p = vb[384:414].reshape(30, NSEC, 128, NGRP).transpose(1, 0, 3, 2)
        in_maps.append({
            "vtp": np.ascontiguousarray(vtp),
            "vt3p": np.ascontiguousarray(vt3p),
            "wt": wt,
        })
    return in_maps


def _assemble(results):
    out = np.empty((NTOK, NOUT), np.float32)
    for i in range(NCORES):
        # ot[p, h, s', q, o] -> token 1024*(2h+s') + 8p + q
        o = results[i]["ot"].astype(np.float32).reshape(128, NSEC, NGRP, NOUT)
        out[i * TPC:(i + 1) * TPC] = (
            o.transpose(1, 0, 2, 3).reshape(TPC, NOUT))
    return out


def kernel(x, init_pose, init_shape, init_cam, fc1_w, fc1_b, fc2_w, fc2_b,
           decshape_w, decshape_b, deccam_w, deccam_b, ktd_w, ktd_b):
    from concourse.bass_utils import run_bass_kernel_spmd

    in_maps = _make_in_maps(x, init_pose, init_shape, init_cam, fc1_w, fc1_b,
                            fc2_w, fc2_b, decshape_w, decshape_b, deccam_w,
                            deccam_b, ktd_w, ktd_b)
    nc = _get_program()
    res = run_bass_kernel_spmd(nc, in_maps, list(range(NCORES)))
    return _assemble(res.results)


# revision 13
# speedup vs baseline: 1.8997x; 1.0895x over previous
"""Trainium2 kernel for nn_HSCR_67396626809127 (gnn_message_passing).

The reference network (fc1/fc2 -> 24-step KTD kinematic-tree recurrence ->
cam/pose/shape heads) contains no nonlinearity (dropout is identity in eval
mode), so the whole module is one affine map:

    out[157] = W @ [x(256) | init_pose(144) | init_shape(10) | init_cam(3)] + b

W [157,413] / b [157] are composed on host in float64 from the small weight
tensors (<5MB total), with the bias folded in as a constant-ones feature row
(K = 414).  The device runs a data-parallel matmul over the B*T = 32768
tokens; each of the 8 cores handles TPC = 4096 tokens.

Precision plan (rel tolerance is 2e-2): activations are quantized to int8
with one scale per feature column, scales folded into the weights; the
SWDGE DMA casts int8 -> bf16 inline (verified exact on HW), the PE runs
bf16 with f32 PSUM accumulation, outputs return as bf16.  Measured rel
err ~1.0e-2, and input HBM traffic halves vs bf16.

Device organization (activations-stationary):
  - stationary lhsT = activation tile [128 feats, 128 tokens] (contiguous,
    so PE fast-weight-load + background weight buffer engage),
    moving rhs = W^T k-chunk [128 feats, 157 outs]
  - psum [128 tokens, 157 outs] accumulates the 4 k-chunks (K = 414
    packed as 3x128 + 30): ~4.9 PE column-streams per token instead of 8.
  - token t of a core maps to (section s, partition p, group q) via
    t = 1024*s + 8*p + q; input DRAM is packed per-section-contiguous
    (3KB int8 DMA descriptors) and the output is stored in (p, half, s', q)
    order so each half-store writes 5KB contiguous runs per partition.
  - int8 input loads ride the gpsimd SWDGE queue (cast during DMA);
    weights/r3 ride HWDGE; output stores use the HWDGE rings, which are
    idle by then.  PSUM->SBUF copies rotate vector/scalar/gpsimd so the
    8 psum banks recycle as fast as the PE refills them.
  - a short burst of warm-up matmuls on a memset tile runs right after
    the engine preamble so the PE HAM throttle reaches full clock before
    the real matmul stream begins.
"""

import numpy as np
import ml_dtypes

ANCESTOR_INDEX = [[], [0], [0], [0], [0, 1], [0, 2], [0, 3], [0, 1, 4],
                  [0, 2, 5], [0, 3, 6], [0, 1, 4, 7], [0, 2, 5, 8],
                  [0, 3, 6, 9], [0, 3, 6, 9], [0, 3, 6, 9], [0, 3, 6, 9, 12],
                  [0, 3, 6, 9, 13], [0, 3, 6, 9, 14], [0, 3, 6, 9, 13, 16],
                  [0, 3, 6, 9, 14, 17], [0, 3, 6, 9, 13, 16, 18],
                  [0, 3, 6, 9, 14, 17, 19], [0, 3, 6, 9, 13, 16, 18, 20],
                  [0, 3, 6, 9, 14, 17, 19, 21]]
HID = 1024
NCORES = 8
B, T = 2048, 16
NTOK = B * T                 # 32768
TPC = NTOK // NCORES         # 4096 tokens per core
NOUT = 157                   # [cam 3 | pose 144 | shape 10]
KV = 414                     # 413 input features + ones row (bias)
NSEC = 4                     # sections of 1024 tokens
NGRP = 8                     # psum groups per section (token = 1024s+8p+q)
NWARM = 5                    # warm-up matmuls (N=512) before the real stream

_PROG = {}


def _compose_affine(fc1_w, fc1_b, fc2_w, fc2_b, decshape_w, decshape_b,
                    deccam_w, deccam_b, ktd_w, ktd_b):
    """Fold the whole network into out = v @ W.T + b, v = [x|pose|shape|cam]."""
    f8 = np.float64
    fc1_w, fc1_b = fc1_w.astype(f8), fc1_b.astype(f8)
    fc2_w, fc2_b = fc2_w.astype(f8), fc2_b.astype(f8)
    decshape_w, decshape_b = decshape_w.astype(f8), decshape_b.astype(f8)
    deccam_w, deccam_b = deccam_w.astype(f8), deccam_b.astype(f8)
    ktd_w, ktd_b = ktd_w.astype(f8), ktd_b.astype(f8)

    F1x, F1s = fc1_w[:, :256], fc1_w[:, 256:266]
    F2x, F2p = fc2_w[:, :256], fc2_w[:, 256:400]

    # KTD recurrence -> pose_out = G @ xc_pose + H @ init_pose + c
    G = np.zeros((24, 6, HID)); H = np.zeros((24, 6, 144)); c = np.zeros((24, 6))
    for j, anc in enumerate(ANCESTOR_INDEX):
        Wj = ktd_w[j]
        G[j] = Wj[:, :HID]
        off = HID
        for i in anc:
            A = Wj[:, off:off + 6]; off += 6
            G[j] += A @ G[i]
            H[j] += A @ H[i]
            c[j] += A @ c[i]
        # reference concatenates init_pose[..., j:j+6] (overlapping slice)
        H[j][:, j:j + 6] += Wj[:, off:off + 6]
        c[j] += ktd_b[j]
    G = G.reshape(144, HID); H = H.reshape(144, 144); c = c.reshape(144)

    Dp, Ds, Dc = deccam_w[:, :HID], deccam_w[:, HID:2 * HID], deccam_w[:, 2 * HID:]

    W = np.zeros((NOUT, 413)); b = np.zeros(NOUT)
    W[0:3, 0:256] = Dp @ F2x + Ds @ F1x
    W[0:3, 256:400] = Dp @ F2p
    W[0:3, 400:410] = Ds @ F1s
    W[0:3, 410:413] = Dc + np.eye(3)
    b[0:3] = Dp @ fc2_b + Ds @ fc1_b + deccam_b

    W[3:147, 0:256] = G @ F2x
    W[3:147, 256:400] = G @ F2p + H + np.eye(144)
    b[3:147] = G @ fc2_b + c

    W[147:157, 0:256] = decshape_w @ F1x
    W[147:157, 400:410] = decshape_w @ F1s + np.eye(10)
    b[147:157] = decshape_w @ fc1_b + decshape_b
    return W.astype(np.float64), b.astype(np.float64)


def _build_program():
    import concourse.bass as bass
    import concourse.tile as tile
    from concourse import bacc, mybir

    f32 = mybir.dt.float32
    bf16 = mybir.dt.bfloat16
    i8 = mybir.dt.int8
    nc = bacc.Bacc("TRN2", target_bir_lowering=False, debug=False,
                   num_devices=NCORES)
    # activations packed per section: vtp[s, f, c, q, p] = feature (c*128+f)
    # of token (1024s + 8p + q), quantized int8 (per-feature scales folded
    # into wt).  lhsT slices [:, k, q, :] are contiguous.
    vtp = nc.declare_dram_parameter("vtp", [NSEC, 128, 3, NGRP, 128], i8,
                                    isOutput=False)
    vt3p = nc.declare_dram_parameter("vt3p", [NSEC, 30, NGRP, 128], bf16,
                                     isOutput=False)
    # W^T packed [128, 4, NOUT]; chunk 3 rows 30..127 are zero (unused)
    wt = nc.declare_dram_parameter("wt", [128, 4, NOUT], bf16, isOutput=False)
    # output in (p, half, s', q, o) order; host un-permutes
    ot = nc.declare_dram_parameter("ot", [128, 2, 2, NGRP, NOUT], bf16,
                                   isOutput=True)

    with tile.TileContext(nc) as tc:
        with (
            tc.tile_pool(name="wpool", bufs=1) as wpool,
            tc.tile_pool(name="rin", bufs=3) as rpool,
            tc.tile_pool(name="outp", bufs=2) as opool,
            tc.tile_pool(name="psum", bufs=1, space=bass.MemorySpace.PSUM) as ppool,
        ):
            # PE warm-up: memset a zeros tile, run a few N=512 matmuls into
            # the ps0 slot so the HAM throttle sees sustained PE activity
            # while the first input DMAs are still in flight.
            z = wpool.tile([128, 512], bf16, tag="z", name="z")
            nc.vector.memset(z[:], 0.0)
            psw = ppool.tile([128, 512], f32, tag="ps0", name="ps_warm")
            for i in range(NWARM):
                nc.tensor.matmul(psw[:], z[:, 0:128], z[:],
                                 start=(i == 0), stop=(i == NWARM - 1))
            zsink = wpool.tile([128, 512], bf16, tag="zsink", name="zsink")
            nc.vector.tensor_copy(zsink[:], psw[:])

            w = wpool.tile([128, 4, NOUT], bf16, tag="w", name="w")
            nc.scalar.dma_start(w[:], wt[:])

            # int8 input loads ride the gpsimd SWDGE queue (cast to bf16
            # during DMA); section 0 split per-chunk for an early PE start
            r012s, r3s = [], []
            r012_0 = rpool.tile([128, 3, NGRP, 128], bf16, tag="r012",
                                name="r012_0")
            nc.gpsimd.dma_start(r012_0[:, 0], vtp[0, :, 0])
            nc.gpsimd.dma_start(r012_0[:, 1:3], vtp[0, :, 1:3])
            r3_0 = rpool.tile([30, NGRP, 128], bf16, tag="r3", name="r3_0")
            nc.sync.dma_start(r3_0[:], vt3p[0])
            r012s.append(r012_0); r3s.append(r3_0)

            def load_section(s):
                r012 = rpool.tile([128, 3, NGRP, 128], bf16, tag="r012",
                                  name=f"r012_{s}")
                nc.gpsimd.dma_start(r012[:], vtp[s])
                r3 = rpool.tile([30, NGRP, 128], bf16, tag="r3", name=f"r3_{s}")
                ring = nc.sync if s % 2 == 1 else nc.scalar
                ring.dma_start(r3[:], vt3p[s])
                r012s.append(r012)
                r3s.append(r3)

            load_section(1)

            cpeng = [None, None, None]

            for s in range(NSEC):
                if s + 1 < NSEC and s > 0:
                    load_section(s + 1)
                r012, r3 = r012s[s], r3s[s]
                pss = []
                for q in range(NGRP):
                    ps = ppool.tile([128, 512], f32, tag=f"ps{q}",
                                    name=f"ps_{s}_{q}")
                    pss.append(ps)
                for k in range(4):
                    rhs = w[:, k, :] if k < 3 else w[0:30, 3, :]
                    for q in range(NGRP):
                        lhsT = r012[:, k, q, :] if k < 3 else r3[:, q, :]
                        nc.tensor.matmul(pss[q][:, 0:NOUT], lhsT, rhs,
                                         start=(k == 0), stop=(k == 3))
                h, sh = divmod(s, 2)
                if sh == 0:
                    outt = opool.tile([128, 2, NGRP, NOUT], bf16, tag="out",
                                      name=f"out_{h}")
                for q in range(NGRP):
                    if q % 2 == 0:
                        nc.vector.tensor_copy(outt[:, sh, q, :],
                                              pss[q][:, 0:NOUT])
                    else:
                        nc.scalar.copy(outt[:, sh, q, :], pss[q][:, 0:NOUT])
                if sh == 1:
                    ring = nc.sync if h == 0 else nc.scalar
                    ring.dma_start(ot[:, h], outt[:])
    nc.compile()
    return nc


def _get_program():
    if "nc" not in _PROG:
        _PROG["nc"] = _build_program()
    return _PROG["nc"]


def _make_in_maps(x, init_pose, init_shape, init_cam, fc1_w, fc1_b, fc2_w,
                  fc2_b, decshape_w, decshape_b, deccam_w, deccam_b, ktd_w,
                  ktd_b):
    bf = ml_dtypes.bfloat16
    x = np.asarray(x, dtype=np.float32)
    init_pose = np.asarray(init_pose, dtype=np.float32)
    init_shape = np.asarray(init_shape, dtype=np.float32)
    init_cam = np.asarray(init_cam, dtype=np.float32)

    W, b = _compose_affine(
        np.asarray(fc1_w), np.asarray(fc1_b), np.asarray(fc2_w),
        np.asarray(fc2_b), np.asarray(decshape_w), np.asarray(decshape_b),
        np.asarray(deccam_w), np.asarray(deccam_b), np.asarray(ktd_w),
        np.asarray(ktd_b))

    # full feature-major activation matrix [414, NTOK]
    v = np.empty((KV, NTOK), np.float32)
    v[0:256] = x.reshape(NTOK, 256).T
    v[256:400] = init_pose.reshape(NTOK, 144).T
    v[400:410] = init_shape.reshape(NTOK, 10).T
    v[410:413] = init_cam.reshape(NTOK, 3).T
    v[413] = 1.0

    # per-feature int8 quantization for features 0..383 (x + pose head);
    # scales folded into the weights.  Features 384..413 stay bf16 raw.
    scale = np.abs(v[0:384]).max(axis=1) / 127.0            # [384]
    q = np.clip(np.round(v[0:384] / scale[:, None]), -127, 127).astype(np.int8)

    W_aug = np.concatenate([W, b[:, None]], axis=1)         # [157, 414] f64
    wtk = W_aug.T.copy()                                    # [414, 157] f64
    wtk[0:384] *= scale[:, None]
    wtk = wtk.astype(np.float32).astype(bf)
    wt = np.zeros((128, 4, NOUT), bf)
    wt[:, 0:3, :] = wtk[0:384].reshape(3, 128, NOUT).transpose(1, 0, 2)
    wt[0:30, 3, :] = wtk[384:414]
    wt = np.ascontiguousarray(wt)

    in_maps = []
    for i in range(NCORES):
        qc = q[:, i * TPC:(i + 1) * TPC]                    # [384, TPC] int8
        # vtp[s, f, c, q, p] = v[c*128+f, 1024s+8p+q]
        vtp = qc.reshape(3, 128, NSEC, 128, NGRP).transpose(2, 1, 0, 4, 3)
        # chunk-3 features stay bf16 (tiny); they skip quantization
        v3 = v[384:414, i * TPC:(i + 1) * TPC].astype(bf)
        vt3p = v3.reshape(30, NSEC, 128, NGRP).transpose(1, 0, 3, 2)
        in_maps.append({
            "vtp": np.ascontiguousarray(vtp),
            "vt3p": np.ascontiguousarray(vt3p),
            "wt": wt,
        })
    return in_maps


def _assemble(results):
    out = np.empty((NTOK, NOUT), np.float32)
    for i in range(NCORES):
        # ot[p, h, s', q, o] -> token 1024*(2h+s') + 8p + q
        o = results[i]["ot"].astype(np.float32).reshape(128, NSEC, NGRP, NOUT)
        out[i * TPC:(i + 1) * TPC] = (
            o.transpose(1, 0, 2, 3).reshape(TPC, NOUT))
    return out


def kernel(x, init_pose, init_shape, init_cam, fc1_w, fc1_b, fc2_w, fc2_b,
           decshape_w, decshape_b, deccam_w, deccam_b, ktd_w, ktd_b):
    from concourse.bass_utils import run_bass_kernel_spmd

    in_maps = _make_in_maps(x, init_pose, init_shape, init_cam, fc1_w, fc1_b,
                            fc2_w, fc2_b, decshape_w, decshape_b, deccam_w,
                            deccam_b, ktd_w, ktd_b)
    nc = _get_program()
    res = run_bass_kernel_spmd(nc, in_maps, list(range(NCORES)))
    return _assemble(res.results)


# revision 16
# speedup vs baseline: 2.0293x; 1.0682x over previous
"""Trainium2 kernel for nn_HSCR_67396626809127 (gnn_message_passing).

The reference network (fc1/fc2 -> 24-step KTD kinematic-tree recurrence ->
cam/pose/shape heads) contains no nonlinearity (dropout is identity in eval
mode), so the whole module is one affine map:

    out[157] = W @ [x(256) | init_pose(144) | init_shape(10) | init_cam(3)] + b

W [157,413] / b [157] are composed on host in float64 from the small weight
tensors (<5MB total), with the bias folded in as a constant-ones feature row
(K = 414).  The device runs a data-parallel matmul over the B*T = 32768
tokens; each of the 8 cores handles TPC = 4096 tokens.

Precision plan (rel tolerance is 2e-2): activations are quantized to int8
with one scale per feature column, scales folded into the weights; the
SWDGE DMA casts int8 -> bf16 inline (verified exact on HW), the PE runs
bf16 with f32 PSUM accumulation, outputs return as bf16.  Measured rel
err ~1.0e-2, and input HBM traffic halves vs bf16.

Device organization (activations-stationary):
  - stationary lhsT = activation tile [128 feats, 128 tokens] (contiguous,
    so PE fast-weight-load + background weight buffer engage),
    moving rhs = W^T k-chunk [128 feats, 157 outs]
  - psum [128 tokens, 157 outs] accumulates the 4 k-chunks (K = 414
    packed as 3x128 + 30): ~4.9 PE column-streams per token instead of 8.
  - token t of a core maps to (section s, partition p, group q) via
    t = 1024*s + 8*p + q; input DRAM is packed per-section-contiguous
    (3KB int8 DMA descriptors) and the output is stored in (p, half, s', q)
    order so each half-store writes 5KB contiguous runs per partition.
  - int8 input loads ride the gpsimd SWDGE queue (cast during DMA);
    weights/r3 ride HWDGE; output stores use the HWDGE rings, which are
    idle by then.  PSUM->SBUF copies rotate vector/scalar/gpsimd so the
    8 psum banks recycle as fast as the PE refills them.
  - a short burst of warm-up matmuls on a memset tile runs right after
    the engine preamble so the PE HAM throttle reaches full clock before
    the real matmul stream begins.
"""

import numpy as np
import ml_dtypes

ANCESTOR_INDEX = [[], [0], [0], [0], [0, 1], [0, 2], [0, 3], [0, 1, 4],
                  [0, 2, 5], [0, 3, 6], [0, 1, 4, 7], [0, 2, 5, 8],
                  [0, 3, 6, 9], [0, 3, 6, 9], [0, 3, 6, 9], [0, 3, 6, 9, 12],
                  [0, 3, 6, 9, 13], [0, 3, 6, 9, 14], [0, 3, 6, 9, 13, 16],
                  [0, 3, 6, 9, 14, 17], [0, 3, 6, 9, 13, 16, 18],
                  [0, 3, 6, 9, 14, 17, 19], [0, 3, 6, 9, 13, 16, 18, 20],
                  [0, 3, 6, 9, 14, 17, 19, 21]]
HID = 1024
NCORES = 8
B, T = 2048, 16
NTOK = B * T                 # 32768
TPC = NTOK // NCORES         # 4096 tokens per core
NOUT = 157                   # [cam 3 | pose 144 | shape 10]
KV = 414                     # 413 input features + ones row (bias)
NSEC = 4                     # sections of 1024 tokens
NGRP = 8                     # psum groups per section (token = 1024s+8p+q)
NWARM = 5                    # warm-up matmuls (N=512) before the real stream

_PROG = {}


def _compose_affine(fc1_w, fc1_b, fc2_w, fc2_b, decshape_w, decshape_b,
                    deccam_w, deccam_b, ktd_w, ktd_b):
    """Fold the whole network into out = v @ W.T + b, v = [x|pose|shape|cam]."""
    f8 = np.float64
    fc1_w, fc1_b = fc1_w.astype(f8), fc1_b.astype(f8)
    fc2_w, fc2_b = fc2_w.astype(f8), fc2_b.astype(f8)
    decshape_w, decshape_b = decshape_w.astype(f8), decshape_b.astype(f8)
    deccam_w, deccam_b = deccam_w.astype(f8), deccam_b.astype(f8)
    ktd_w, ktd_b = ktd_w.astype(f8), ktd_b.astype(f8)

    F1x, F1s = fc1_w[:, :256], fc1_w[:, 256:266]
    F2x, F2p = fc2_w[:, :256], fc2_w[:, 256:400]

    # KTD recurrence -> pose_out = G @ xc_pose + H @ init_pose + c
    G = np.zeros((24, 6, HID)); H = np.zeros((24, 6, 144)); c = np.zeros((24, 6))
    for j, anc in enumerate(ANCESTOR_INDEX):
        Wj = ktd_w[j]
        G[j] = Wj[:, :HID]
        off = HID
        for i in anc:
            A = Wj[:, off:off + 6]; off += 6
            G[j] += A @ G[i]
            H[j] += A @ H[i]
            c[j] += A @ c[i]
        # reference concatenates init_pose[..., j:j+6] (overlapping slice)
        H[j][:, j:j + 6] += Wj[:, off:off + 6]
        c[j] += ktd_b[j]
    G = G.reshape(144, HID); H = H.reshape(144, 144); c = c.reshape(144)

    Dp, Ds, Dc = deccam_w[:, :HID], deccam_w[:, HID:2 * HID], deccam_w[:, 2 * HID:]

    W = np.zeros((NOUT, 413)); b = np.zeros(NOUT)
    W[0:3, 0:256] = Dp @ F2x + Ds @ F1x
    W[0:3, 256:400] = Dp @ F2p
    W[0:3, 400:410] = Ds @ F1s
    W[0:3, 410:413] = Dc + np.eye(3)
    b[0:3] = Dp @ fc2_b + Ds @ fc1_b + deccam_b

    W[3:147, 0:256] = G @ F2x
    W[3:147, 256:400] = G @ F2p + H + np.eye(144)
    b[3:147] = G @ fc2_b + c

    W[147:157, 0:256] = decshape_w @ F1x
    W[147:157, 400:410] = decshape_w @ F1s + np.eye(10)
    b[147:157] = decshape_w @ fc1_b + decshape_b
    return W.astype(np.float64), b.astype(np.float64)


def _build_program():
    import concourse.bass as bass
    import concourse.tile as tile
    from concourse import bacc, mybir

    f32 = mybir.dt.float32
    bf16 = mybir.dt.bfloat16
    i8 = mybir.dt.int8
    nc = bacc.Bacc("TRN2", target_bir_lowering=False, debug=False,
                   num_devices=NCORES)
    # activations packed per section: vtp[s, f, c, q, p] = feature (c*128+f)
    # of token (1024s + 8p + q), quantized int8 (per-feature scales folded
    # into wt).  lhsT slices [:, k, q, :] are contiguous.
    vtp = nc.declare_dram_parameter("vtp", [NSEC, 128, 3, NGRP, 128], i8,
                                    isOutput=False)
    vt3p = nc.declare_dram_parameter("vt3p", [NSEC, 30, NGRP, 128], bf16,
                                     isOutput=False)
    # W^T packed [128, 4, NOUT]; chunk 3 rows 30..127 are zero (unused)
    wt = nc.declare_dram_parameter("wt", [128, 4, NOUT], bf16, isOutput=False)
    # output in (p, half, s', q, o) order; host un-permutes
    ot = nc.declare_dram_parameter("ot", [128, 2, 2, NGRP, NOUT], bf16,
                                   isOutput=True)

    with tile.TileContext(nc) as tc:
        with (
            tc.tile_pool(name="wpool", bufs=1) as wpool,
            tc.tile_pool(name="rin", bufs=3) as rpool,
            tc.tile_pool(name="outp", bufs=3) as opool,
            tc.tile_pool(name="psum", bufs=1, space=bass.MemorySpace.PSUM) as ppool,
        ):
            # PE warm-up: memset a zeros tile, run a few N=512 matmuls into
            # the ps0 slot so the HAM throttle sees sustained PE activity
            # while the first input DMAs are still in flight.
            z = wpool.tile([128, 512], bf16, tag="z", name="z")
            nc.vector.memset(z[:], 0.0)
            psw = ppool.tile([128, 512], f32, tag="ps0", name="ps_warm")
            for i in range(NWARM):
                nc.tensor.matmul(psw[:], z[:, 0:128], z[:],
                                 start=(i == 0), stop=(i == NWARM - 1))
            zsink = wpool.tile([128, 512], bf16, tag="zsink", name="zsink")
            nc.vector.tensor_copy(zsink[:], psw[:])

            w = wpool.tile([128, 4, NOUT], bf16, tag="w", name="w")
            nc.scalar.dma_start(w[:], wt[:])

            # int8 input loads ride the gpsimd SWDGE queue (cast to bf16
            # during DMA); section 0 split per-chunk for an early PE start
            r012s, r3s = [], []
            r012_0 = rpool.tile([128, 3, NGRP, 128], bf16, tag="r012",
                                name="r012_0")
            nc.gpsimd.dma_start(r012_0[:, 0], vtp[0, :, 0])
            nc.gpsimd.dma_start(r012_0[:, 1], vtp[0, :, 1])
            nc.gpsimd.dma_start(r012_0[:, 2], vtp[0, :, 2])
            r3_0 = rpool.tile([30, NGRP, 128], bf16, tag="r3", name="r3_0")
            nc.sync.dma_start(r3_0[:], vt3p[0])
            r012s.append(r012_0); r3s.append(r3_0)

            def load_section(s):
                r012 = rpool.tile([128, 3, NGRP, 128], bf16, tag="r012",
                                  name=f"r012_{s}")
                nc.gpsimd.dma_start(r012[:], vtp[s])
                r3 = rpool.tile([30, NGRP, 128], bf16, tag="r3", name=f"r3_{s}")
                ring = nc.sync if s % 2 == 1 else nc.scalar
                ring.dma_start(r3[:], vt3p[s])
                r012s.append(r012)
                r3s.append(r3)

            load_section(1)

            cpeng = [None, None, None]

            for s in range(NSEC):
                if s + 1 < NSEC and s > 0:
                    load_section(s + 1)
                r012, r3 = r012s[s], r3s[s]
                pss = []
                for q in range(NGRP):
                    ps = ppool.tile([128, 512], f32, tag=f"ps{q}",
                                    name=f"ps_{s}_{q}")
                    pss.append(ps)
                for k in range(4):
                    rhs = w[:, k, :] if k < 3 else w[0:30, 3, :]
                    for q in range(NGRP):
                        lhsT = r012[:, k, q, :] if k < 3 else r3[:, q, :]
                        nc.tensor.matmul(pss[q][:, 0:NOUT], lhsT, rhs,
                                         start=(k == 0), stop=(k == 3))
                h, sh = divmod(s, 2)
                outt = opool.tile([128, NGRP, NOUT], bf16, tag="out",
                                  name=f"out_{s}")
                for q in range(NGRP):
                    if q % 2 == 0:
                        nc.vector.tensor_copy(outt[:, q, :],
                                              pss[q][:, 0:NOUT])
                    else:
                        nc.scalar.copy(outt[:, q, :], pss[q][:, 0:NOUT])
                ring = nc.sync if s % 2 == 0 else nc.scalar
                ring.dma_start(ot[:, h, sh], outt[:])
    nc.compile()
    return nc


def _get_program():
    if "nc" not in _PROG:
        _PROG["nc"] = _build_program()
    return _PROG["nc"]


def _make_in_maps(x, init_pose, init_shape, init_cam, fc1_w, fc1_b, fc2_w,
                  fc2_b, decshape_w, decshape_b, deccam_w, deccam_b, ktd_w,
                  ktd_b):
    bf = ml_dtypes.bfloat16
    x = np.asarray(x, dtype=np.float32)
    init_pose = np.asarray(init_pose, dtype=np.float32)
    init_shape = np.asarray(init_shape, dtype=np.float32)
    init_cam = np.asarray(init_cam, dtype=np.float32)

    W, b = _compose_affine(
        np.asarray(fc1_w), np.asarray(fc1_b), np.asarray(fc2_w),
        np.asarray(fc2_b), np.asarray(decshape_w), np.asarray(decshape_b),
        np.asarray(deccam_w), np.asarray(deccam_b), np.asarray(ktd_w),
        np.asarray(ktd_b))

    # full feature-major activation matrix [414, NTOK]
    v = np.empty((KV, NTOK), np.float32)
    v[0:256] = x.reshape(NTOK, 256).T
    v[256:400] = init_pose.reshape(NTOK, 144).T
    v[400:410] = init_shape.reshape(NTOK, 10).T
    v[410:413] = init_cam.reshape(NTOK, 3).T
    v[413] = 1.0

    # per-feature int8 quantization for features 0..383 (x + pose head);
    # scales folded into the weights.  Features 384..413 stay bf16 raw.
    scale = np.abs(v[0:384]).max(axis=1) / 127.0            # [384]
    q = np.clip(np.round(v[0:384] / scale[:, None]), -127, 127).astype(np.int8)

    W_aug = np.concatenate([W, b[:, None]], axis=1)         # [157, 414] f64
    wtk = W_aug.T.copy()                                    # [414, 157] f64
    wtk[0:384] *= scale[:, None]
    wtk = wtk.astype(np.float32).astype(bf)
    wt = np.zeros((128, 4, NOUT), bf)
    wt[:, 0:3, :] = wtk[0:384].reshape(3, 128, NOUT).transpose(1, 0, 2)
    wt[0:30, 3, :] = wtk[384:414]
    wt = np.ascontiguousarray(wt)

    in_maps = []
    for i in range(NCORES):
        qc = q[:, i * TPC:(i + 1) * TPC]                    # [384, TPC] int8
        # vtp[s, f, c, q, p] = v[c*128+f, 1024s+8p+q]
        vtp = qc.reshape(3, 128, NSEC, 128, NGRP).transpose(2, 1, 0, 4, 3)
        # chunk-3 features stay bf16 (tiny); they skip quantization
        v3 = v[384:414, i * TPC:(i + 1) * TPC].astype(bf)
        vt3p = v3.reshape(30, NSEC, 128, NGRP).transpose(1, 0, 3, 2)
        in_maps.append({
            "vtp": np.ascontiguousarray(vtp),
            "vt3p": np.ascontiguousarray(vt3p),
            "wt": wt,
        })
    return in_maps


def _assemble(results):
    out = np.empty((NTOK, NOUT), np.float32)
    for i in range(NCORES):
        # ot[p, h, s', q, o] -> token 1024*(2h+s') + 8p + q
        o = results[i]["ot"].astype(np.float32).reshape(128, NSEC, NGRP, NOUT)
        out[i * TPC:(i + 1) * TPC] = (
            o.transpose(1, 0, 2, 3).reshape(TPC, NOUT))
    return out


def kernel(x, init_pose, init_shape, init_cam, fc1_w, fc1_b, fc2_w, fc2_b,
           decshape_w, decshape_b, deccam_w, deccam_b, ktd_w, ktd_b):
    from concourse.bass_utils import run_bass_kernel_spmd

    in_maps = _make_in_maps(x, init_pose, init_shape, init_cam, fc1_w, fc1_b,
                            fc2_w, fc2_b, decshape_w, decshape_b, deccam_w,
                            deccam_b, ktd_w, ktd_b)
    nc = _get_program()
    res = run_bass_kernel_spmd(nc, in_maps, list(range(NCORES)))
    return _assemble(res.results)


# revision 17
# speedup vs baseline: 2.1522x; 1.0606x over previous
"""Trainium2 kernel for nn_HSCR_67396626809127 (gnn_message_passing).

The reference network (fc1/fc2 -> 24-step KTD kinematic-tree recurrence ->
cam/pose/shape heads) contains no nonlinearity (dropout is identity in eval
mode), so the whole module is one affine map:

    out[157] = W @ [x(256) | init_pose(144) | init_shape(10) | init_cam(3)] + b

W [157,413] / b [157] are composed on host in float64 from the small weight
tensors (<5MB total), with the bias folded in as a constant-ones feature row
(K = 414).  The device runs a data-parallel matmul over the B*T = 32768
tokens; each of the 8 cores handles TPC = 4096 tokens.

Precision plan (rel tolerance is 2e-2): activations are quantized to int8
with one scale per feature column, scales folded into the weights; the
SWDGE DMA casts int8 -> bf16 inline (verified exact on HW), the PE runs
bf16 with f32 PSUM accumulation, outputs return as bf16.  Measured rel
err ~1.0e-2, and input HBM traffic halves vs bf16.

Device organization (activations-stationary):
  - stationary lhsT = activation tile [128 feats, 128 tokens] (contiguous,
    so PE fast-weight-load + background weight buffer engage),
    moving rhs = W^T k-chunk [128 feats, 157 outs]
  - psum [128 tokens, 157 outs] accumulates the 4 k-chunks (K = 414
    packed as 3x128 + 30): ~4.9 PE column-streams per token instead of 8.
  - token t of a core maps to (section s, partition p, group q) via
    t = 1024*s + 8*p + q; input DRAM is packed per-section-contiguous
    (3KB int8 DMA descriptors) and the output is stored in (p, half, s', q)
    order so each half-store writes 5KB contiguous runs per partition.
  - int8 input loads ride the gpsimd SWDGE queue (cast during DMA);
    weights/r3 ride HWDGE; output stores use the HWDGE rings, which are
    idle by then.  PSUM->SBUF copies rotate vector/scalar/gpsimd so the
    8 psum banks recycle as fast as the PE refills them.
  - a short burst of warm-up matmuls on a memset tile runs right after
    the engine preamble so the PE HAM throttle reaches full clock before
    the real matmul stream begins.
"""

import numpy as np
import ml_dtypes

ANCESTOR_INDEX = [[], [0], [0], [0], [0, 1], [0, 2], [0, 3], [0, 1, 4],
                  [0, 2, 5], [0, 3, 6], [0, 1, 4, 7], [0, 2, 5, 8],
                  [0, 3, 6, 9], [0, 3, 6, 9], [0, 3, 6, 9], [0, 3, 6, 9, 12],
                  [0, 3, 6, 9, 13], [0, 3, 6, 9, 14], [0, 3, 6, 9, 13, 16],
                  [0, 3, 6, 9, 14, 17], [0, 3, 6, 9, 13, 16, 18],
                  [0, 3, 6, 9, 14, 17, 19], [0, 3, 6, 9, 13, 16, 18, 20],
                  [0, 3, 6, 9, 14, 17, 19, 21]]
HID = 1024
NCORES = 8
B, T = 2048, 16
NTOK = B * T                 # 32768
TPC = NTOK // NCORES         # 4096 tokens per core
NOUT = 157                   # [cam 3 | pose 144 | shape 10]
KV = 414                     # 413 input features + ones row (bias)
NSEC = 4                     # sections of 1024 tokens
NGRP = 8                     # psum groups per section (token = 1024s+8p+q)
NWARM = 3                    # warm-up matmuls (N=512) before the real stream

_PROG = {}


def _compose_affine(fc1_w, fc1_b, fc2_w, fc2_b, decshape_w, decshape_b,
                    deccam_w, deccam_b, ktd_w, ktd_b):
    """Fold the whole network into out = v @ W.T + b, v = [x|pose|shape|cam]."""
    f8 = np.float64
    fc1_w, fc1_b = fc1_w.astype(f8), fc1_b.astype(f8)
    fc2_w, fc2_b = fc2_w.astype(f8), fc2_b.astype(f8)
    decshape_w, decshape_b = decshape_w.astype(f8), decshape_b.astype(f8)
    deccam_w, deccam_b = deccam_w.astype(f8), deccam_b.astype(f8)
    ktd_w, ktd_b = ktd_w.astype(f8), ktd_b.astype(f8)

    F1x, F1s = fc1_w[:, :256], fc1_w[:, 256:266]
    F2x, F2p = fc2_w[:, :256], fc2_w[:, 256:400]

    # KTD recurrence -> pose_out = G @ xc_pose + H @ init_pose + c
    G = np.zeros((24, 6, HID)); H = np.zeros((24, 6, 144)); c = np.zeros((24, 6))
    for j, anc in enumerate(ANCESTOR_INDEX):
        Wj = ktd_w[j]
        G[j] = Wj[:, :HID]
        off = HID
        for i in anc:
            A = Wj[:, off:off + 6]; off += 6
            G[j] += A @ G[i]
            H[j] += A @ H[i]
            c[j] += A @ c[i]
        # reference concatenates init_pose[..., j:j+6] (overlapping slice)
        H[j][:, j:j + 6] += Wj[:, off:off + 6]
        c[j] += ktd_b[j]
    G = G.reshape(144, HID); H = H.reshape(144, 144); c = c.reshape(144)

    Dp, Ds, Dc = deccam_w[:, :HID], deccam_w[:, HID:2 * HID], deccam_w[:, 2 * HID:]

    W = np.zeros((NOUT, 413)); b = np.zeros(NOUT)
    W[0:3, 0:256] = Dp @ F2x + Ds @ F1x
    W[0:3, 256:400] = Dp @ F2p
    W[0:3, 400:410] = Ds @ F1s
    W[0:3, 410:413] = Dc + np.eye(3)
    b[0:3] = Dp @ fc2_b + Ds @ fc1_b + deccam_b

    W[3:147, 0:256] = G @ F2x
    W[3:147, 256:400] = G @ F2p + H + np.eye(144)
    b[3:147] = G @ fc2_b + c

    W[147:157, 0:256] = decshape_w @ F1x
    W[147:157, 400:410] = decshape_w @ F1s + np.eye(10)
    b[147:157] = decshape_w @ fc1_b + decshape_b
    return W.astype(np.float64), b.astype(np.float64)


def _build_program():
    import concourse.bass as bass
    import concourse.tile as tile
    from concourse import bacc, mybir

    f32 = mybir.dt.float32
    bf16 = mybir.dt.bfloat16
    i8 = mybir.dt.int8
    nc = bacc.Bacc("TRN2", target_bir_lowering=False, debug=False,
                   num_devices=NCORES)
    # activations packed per section: vtp[s, f, c, q, p] = feature (c*128+f)
    # of token (1024s + 8p + q), quantized int8 (per-feature scales folded
    # into wt).  lhsT slices [:, k, q, :] are contiguous.
    vtp = nc.declare_dram_parameter("vtp", [NSEC, 128, 3, NGRP, 128], i8,
                                    isOutput=False)
    vt3p = nc.declare_dram_parameter("vt3p", [NSEC, 30, NGRP, 128], bf16,
                                     isOutput=False)
    # bf16 copy of section 0 / chunk 0 so the very first lhsT tile can ride
    # the faster HWDGE path (SWDGE cast DMAs have ~1us extra first-byte)
    vt0b = nc.declare_dram_parameter("vt0b", [128, NGRP, 128], bf16,
                                     isOutput=False)
    # W^T packed [128, 4, NOUT]; chunk 3 rows 30..127 are zero (unused)
    wt = nc.declare_dram_parameter("wt", [128, 4, NOUT], bf16, isOutput=False)
    # output in (p, half, s', q, o) order; host un-permutes
    ot = nc.declare_dram_parameter("ot", [128, 2, 2, NGRP, NOUT], bf16,
                                   isOutput=True)

    with tile.TileContext(nc) as tc:
        with (
            tc.tile_pool(name="wpool", bufs=1) as wpool,
            tc.tile_pool(name="rin", bufs=3) as rpool,
            tc.tile_pool(name="outp", bufs=3) as opool,
            tc.tile_pool(name="psum", bufs=1, space=bass.MemorySpace.PSUM) as ppool,
        ):
            # PE warm-up: memset a zeros tile, run a few N=512 matmuls into
            # the ps0 slot so the HAM throttle sees sustained PE activity
            # while the first input DMAs are still in flight.
            z = wpool.tile([128, 512], bf16, tag="z", name="z")
            nc.vector.memset(z[:], 0.0)
            psw = ppool.tile([128, 512], f32, tag="ps0", name="ps_warm")
            for i in range(NWARM):
                nc.tensor.matmul(psw[:], z[:, 0:128], z[:],
                                 start=(i == 0), stop=(i == NWARM - 1))
            zsink = wpool.tile([128, 512], bf16, tag="zsink", name="zsink")
            nc.vector.tensor_copy(zsink[:], psw[:])

            w = wpool.tile([128, 4, NOUT], bf16, tag="w", name="w")
            nc.scalar.dma_start(w[:], wt[:])

            # int8 input loads ride the gpsimd SWDGE queue (cast to bf16
            # during DMA); section 0 split per-chunk for an early PE start
            r012s, r3s = [], []
            r012_0 = rpool.tile([128, 3, NGRP, 128], bf16, tag="r012",
                                name="r012_0")
            nc.sync.dma_start(r012_0[:, 0], vt0b[:])
            nc.gpsimd.dma_start(r012_0[:, 1], vtp[0, :, 1])
            nc.gpsimd.dma_start(r012_0[:, 2], vtp[0, :, 2])
            r3_0 = rpool.tile([30, NGRP, 128], bf16, tag="r3", name="r3_0")
            nc.sync.dma_start(r3_0[:], vt3p[0])
            r012s.append(r012_0); r3s.append(r3_0)

            def load_section(s):
                r012 = rpool.tile([128, 3, NGRP, 128], bf16, tag="r012",
                                  name=f"r012_{s}")
                nc.gpsimd.dma_start(r012[:], vtp[s])
                r3 = rpool.tile([30, NGRP, 128], bf16, tag="r3", name=f"r3_{s}")
                ring = nc.sync if s % 2 == 1 else nc.scalar
                ring.dma_start(r3[:], vt3p[s])
                r012s.append(r012)
                r3s.append(r3)

            load_section(1)

            cpeng = [None, None, None]

            for s in range(NSEC):
                if s + 1 < NSEC and s > 0:
                    load_section(s + 1)
                r012, r3 = r012s[s], r3s[s]
                pss = []
                for q in range(NGRP):
                    ps = ppool.tile([128, 512], f32, tag=f"ps{q}",
                                    name=f"ps_{s}_{q}")
                    pss.append(ps)
                for k in range(4):
                    rhs = w[:, k, :] if k < 3 else w[0:30, 3, :]
                    for q in range(NGRP):
                        lhsT = r012[:, k, q, :] if k < 3 else r3[:, q, :]
                        nc.tensor.matmul(pss[q][:, 0:NOUT], lhsT, rhs,
                                         start=(k == 0), stop=(k == 3))
                h, sh = divmod(s, 2)
                outt = opool.tile([128, NGRP, NOUT], bf16, tag="out",
                                  name=f"out_{s}")
                for q in range(NGRP):
                    if q % 2 == 0:
                        nc.vector.tensor_copy(outt[:, q, :],
                                              pss[q][:, 0:NOUT])
                    else:
                        nc.scalar.copy(outt[:, q, :], pss[q][:, 0:NOUT])
                if s == NSEC - 1:
                    nc.sync.dma_start(ot[:, h, sh, 0:4], outt[:, 0:4])
                    nc.scalar.dma_start(ot[:, h, sh, 4:8], outt[:, 4:8])
                else:
                    ring = nc.sync if s % 2 == 0 else nc.scalar
                    ring.dma_start(ot[:, h, sh], outt[:])
    nc.compile()
    return nc


def _get_program():
    if "nc" not in _PROG:
        _PROG["nc"] = _build_program()
    return _PROG["nc"]


def _make_in_maps(x, init_pose, init_shape, init_cam, fc1_w, fc1_b, fc2_w,
                  fc2_b, decshape_w, decshape_b, deccam_w, deccam_b, ktd_w,
                  ktd_b):
    bf = ml_dtypes.bfloat16
    x = np.asarray(x, dtype=np.float32)
    init_pose = np.asarray(init_pose, dtype=np.float32)
    init_shape = np.asarray(init_shape, dtype=np.float32)
    init_cam = np.asarray(init_cam, dtype=np.float32)

    W, b = _compose_affine(
        np.asarray(fc1_w), np.asarray(fc1_b), np.asarray(fc2_w),
        np.asarray(fc2_b), np.asarray(decshape_w), np.asarray(decshape_b),
        np.asarray(deccam_w), np.asarray(deccam_b), np.asarray(ktd_w),
        np.asarray(ktd_b))

    # full feature-major activation matrix [414, NTOK]
    v = np.empty((KV, NTOK), np.float32)
    v[0:256] = x.reshape(NTOK, 256).T
    v[256:400] = init_pose.reshape(NTOK, 144).T
    v[400:410] = init_shape.reshape(NTOK, 10).T
    v[410:413] = init_cam.reshape(NTOK, 3).T
    v[413] = 1.0

    # per-feature int8 quantization for features 0..383 (x + pose head);
    # scales folded into the weights.  Features 384..413 stay bf16 raw.
    scale = np.abs(v[0:384]).max(axis=1) / 127.0            # [384]
    q = np.clip(np.round(v[0:384] / scale[:, None]), -127, 127).astype(np.int8)

    W_aug = np.concatenate([W, b[:, None]], axis=1)         # [157, 414] f64
    wtk = W_aug.T.copy()                                    # [414, 157] f64
    wtk[0:384] *= scale[:, None]
    wtk = wtk.astype(np.float32).astype(bf)
    wt = np.zeros((128, 4, NOUT), bf)
    wt[:, 0:3, :] = wtk[0:384].reshape(3, 128, NOUT).transpose(1, 0, 2)
    wt[0:30, 3, :] = wtk[384:414]
    wt = np.ascontiguousarray(wt)

    in_maps = []
    for i in range(NCORES):
        qc = q[:, i * TPC:(i + 1) * TPC]                    # [384, TPC] int8
        # vtp[s, f, c, q, p] = v[c*128+f, 1024s+8p+q]
        vtp = qc.reshape(3, 128, NSEC, 128, NGRP).transpose(2, 1, 0, 4, 3)
        # chunk-3 features stay bf16 (tiny); they skip quantization
        v3 = v[384:414, i * TPC:(i + 1) * TPC].astype(bf)
        vt3p = v3.reshape(30, NSEC, 128, NGRP).transpose(1, 0, 3, 2)
        vtp = np.ascontiguousarray(vtp)
        in_maps.append({
            "vtp": vtp,
            "vt3p": np.ascontiguousarray(vt3p),
            "vt0b": vtp[0][:, 0].astype(bf),
            "wt": wt,
        })
    return in_maps


def _assemble(results):
    out = np.empty((NTOK, NOUT), np.float32)
    for i in range(NCORES):
        # ot[p, h, s', q, o] -> token 1024*(2h+s') + 8p + q
        o = results[i]["ot"].astype(np.float32).reshape(128, NSEC, NGRP, NOUT)
        out[i * TPC:(i + 1) * TPC] = (
            o.transpose(1, 0, 2, 3).reshape(TPC, NOUT))
    return out


def kernel(x, init_pose, init_shape, init_cam, fc1_w, fc1_b, fc2_w, fc2_b,
           decshape_w, decshape_b, deccam_w, deccam_b, ktd_w, ktd_b):
    from concourse.bass_utils import run_bass_kernel_spmd

    in_maps = _make_in_maps(x, init_pose, init_shape, init_cam, fc1_w, fc1_b,
                            fc2_w, fc2_b, decshape_w, decshape_b, deccam_w,
                            deccam_b, ktd_w, ktd_b)
    nc = _get_program()
    res = run_bass_kernel_spmd(nc, in_maps, list(range(NCORES)))
    return _assemble(res.results)
